# revision 1
# baseline (speedup 1.0000x reference)
"""Trainium2 Bass kernel for nn_BERTClassifier (batch-mixing attention BERT).

Key observation: the reference returns `x[0] @ Wc + bc` where every op in the
network is independent across the sequence dim (attention mixes the *batch*
within one position). So the output depends only on position 0: a [32, 768]
activation through 4 layers. The work is dominated by streaming the 113MB of
weights from HBM, so we use 8-way tensor parallelism:

- Feature dim (768) is sharded as 6 x 128 across cores 0..5 (cores 6,7 carry
  duplicate shards; their gather chunks are ignored). FFN dim 3072 is sharded
  6 x 512.
- Residual stream is kept feature-major (xT: [128 partitions, 6 chunks, 32
  tokens]) so LayerNorm stats come from PE column-sums and no transposes are
  needed on the residual path.
- 3 AllGathers per layer: (QT,KT,V) pack, hT pack, o2T pack. Wo is replicated
  (full) on each core, so the attention output projection needs no collective.
- Weights are cast fp32->fp16 inside the HBM->SBUF DMA (gpsimd SWDGE casting
  DMA: zero engine cost) because fp32 matmuls stream at 1/4 rate on the PE.
  Matmuls run fp16 with fp32 PSUM accumulation; softmax stats, LayerNorm and
  the residual stream stay fp32. Weights streamed per-layer, double-buffered.
- A tiny warmup AllGather absorbs the ncfw init + rank-arrival skew under the
  weight DMAs.

Self-contained: shapes hardcoded, no sibling imports.
"""
import os
import sys
import types

import numpy as np

# If BASS_TRACE is set but the axon NTFF hook module is absent, bass_utils
# would crash importing antenv.axon_hooks. Provide a null hook so tracing
# degrades to a warning instead. (test.py installs the real hook first.)
try:
    from antenv import axon_hooks as _ah  # noqa: F401
except ImportError:
    try:
        import antenv as _antenv
        _mod = types.ModuleType("antenv.axon_hooks")
        _mod.get_axon_ntff_profile_hook = lambda: None
        _mod.set_axon_ntff_profile_hook = lambda h: None
        _antenv.axon_hooks = _mod
        sys.modules["antenv.axon_hooks"] = _mod
    except Exception:
        pass

import concourse.bass as bass
import concourse.bacc as bacc
import concourse.mybir as mybir
import concourse.tile as tile
from concourse import masks
from concourse.bass_utils import run_bass_kernel_spmd

F32 = mybir.dt.float32
F16 = mybir.dt.float16
F32R = mybir.dt.float32r
AX = mybir.AxisListType
ALU = mybir.AluOpType
ACT_F = mybir.ActivationFunctionType

V, E, F, L, S, B, C = 30522, 768, 3072, 4, 512, 32, 2
NC = 8            # cores
NSH = 6           # real shard ranks (cores 6,7 duplicate)
ESH = E // NSH    # 128 feature shard
FSH = F // NSH    # 512 ffn shard
KC = E // 128     # 6 contraction chunks of 128
SCALE = 1.0 / float(np.sqrt(E))
EPS = 1e-5

_CACHE = {}
LAST_RESULT = None  # BassKernelResults of the most recent run (for test.py)


def _declare(nc, use_bias, use_affine):
    h = {}
    h["embT"] = nc.dram_tensor("embT", [E, B], F32, kind="ExternalInput")
    h["posT"] = nc.dram_tensor("posT", [E, B], F32, kind="ExternalInput")
    for l in range(L):
        h[f"wqkv{l}"] = nc.dram_tensor(f"wqkv{l}", [E, 3 * ESH], F32, kind="ExternalInput")
        h[f"wo{l}"] = nc.dram_tensor(f"wo{l}", [E, E], F32, kind="ExternalInput")
        h[f"w1{l}"] = nc.dram_tensor(f"w1{l}", [E, FSH], F32, kind="ExternalInput")
        h[f"w2{l}"] = nc.dram_tensor(f"w2{l}", [F, ESH], F32, kind="ExternalInput")
        if use_bias:
            h[f"bqkv{l}"] = nc.dram_tensor(f"bqkv{l}", [B, 3 * ESH], F32, kind="ExternalInput")
            h[f"bo{l}"] = nc.dram_tensor(f"bo{l}", [E, 1], F32, kind="ExternalInput")
            h[f"bf1{l}"] = nc.dram_tensor(f"bf1{l}", [FSH, 1], F32, kind="ExternalInput")
            h[f"bf2{l}"] = nc.dram_tensor(f"bf2{l}", [E, 1], F32, kind="ExternalInput")
        if use_affine:
            h[f"g1{l}"] = nc.dram_tensor(f"g1{l}", [E, 1], F32, kind="ExternalInput")
            h[f"be1{l}"] = nc.dram_tensor(f"be1{l}", [E, 1], F32, kind="ExternalInput")
            h[f"g2{l}"] = nc.dram_tensor(f"g2{l}", [E, 1], F32, kind="ExternalInput")
            h[f"be2{l}"] = nc.dram_tensor(f"be2{l}", [E, 1], F32, kind="ExternalInput")
    h["wc"] = nc.dram_tensor("wc", [E, C], F32, kind="ExternalInput")
    if use_bias:
        h["bc"] = nc.dram_tensor("bc", [B, C], F32, kind="ExternalInput")
    h["out"] = nc.dram_tensor("out", [B, C], F32, kind="ExternalOutput")
    return h


def _emit(tc, h, use_bias, use_affine):
    nc = tc.nc
    groups = [list(range(NC))]
    ctxs = []

    def pool(*a, **k):
        p = tc.alloc_tile_pool(*a, **k)
        ctxs.append(p)
        return p

    const = pool(name="const", bufs=1)
    wp = pool(name="wts", bufs=2)
    ab = pool(name="act", bufs=2)
    ps = pool(name="ps", bufs=2, space="PSUM")
    dr = pool(name="dram", bufs=2, space="DRAM")

    ones_col = const.tile([128, 1], F32)
    nc.vector.memset(ones_col[:], 1.0)
    ones_row = const.tile([1, 128], F32)
    nc.vector.memset(ones_row[:], 1.0)
    eps_sb = const.tile([1, 1], F32)
    nc.vector.memset(eps_sb[:], EPS)
    ident = const.tile([B, B], F32)
    masks.make_identity(nc, ident[:])
    ident16 = const.tile([B, B], F16)
    masks.make_identity(nc, ident16[:])

    # ---- embedding: xT = embT + posT, feature-major [128, 6, 32]
    embT_sb = ab.tile([128, KC, B], F32, tag="emb")
    posT_sb = ab.tile([128, KC, B], F32, tag="pos")
    nc.sync.dma_start(embT_sb[:], h["embT"].ap().rearrange("(k p) b -> p k b", p=128))
    nc.sync.dma_start(posT_sb[:], h["posT"].ap().rearrange("(k p) b -> p k b", p=128))
    xT = ab.tile([128, KC, B], F32, tag="xt")
    nc.vector.tensor_tensor(xT[:], embT_sb[:], posT_sb[:], op=ALU.add)

    def load_w(name, shape_kn, dt=F32):
        # [rows, cols] DRAM -> [128, rows//128, cols] SBUF. For 16-bit dt the
        # gpsimd (SWDGE) DMA casts fp32->fp16 in the datapath: no engine cost.
        t = wp.tile([128, shape_kn[0] // 128, shape_kn[1]], dt, tag=name[:2])
        src_ap = h[name].ap().rearrange("(k p) n -> p k n", p=128)
        if dt == F32:
            nc.sync.dma_start(t[:], src_ap)
        else:
            nc.gpsimd.dma_start(t[:], src_ap)
        return t

    def block_transpose(dst, src, nblk_out, width, dt=None):
        # src: [32, width] sbuf (token-major); dst: [128, nblk_out, 32]
        # (feature-major), width = nblk_out*128. PE transposes per 128-chunk.
        idt = ident16 if dt == F16 else ident
        for j in range(nblk_out):
            t_ps = ps.tile([128, B], dt or F32, tag="qk")
            nc.tensor.transpose(t_ps[:], src[:, 128 * j:128 * (j + 1)], idt[:])
            nc.vector.tensor_copy(dst[:, j, :], t_ps[:])

    def layernorm(yT, g=None, be=None):
        sq = ab.tile([128, KC, B], F32, tag="sq")
        nc.vector.tensor_tensor(sq[:], yT[:], yT[:], op=ALU.mult)
        s_ps = ps.tile([1, KC, B], F32, tag="ln")
        s2_ps = ps.tile([1, KC, B], F32, tag="ln")
        nc.tensor.matmul(s_ps[:], ones_col[:], yT[:], start=True, stop=True)
        nc.tensor.matmul(s2_ps[:], ones_col[:], sq[:], start=True, stop=True)
        mean = ab.tile([1, B], F32, tag="mean")
        nc.vector.tensor_reduce(
            mean[:], s_ps[:].rearrange("p k b -> p b k"), axis=AX.X, op=ALU.add)
        nc.vector.tensor_scalar_mul(mean[:], mean[:], 1.0 / E)
        ex2 = ab.tile([1, B], F32, tag="ex2")
        nc.vector.tensor_reduce(
            ex2[:], s2_ps[:].rearrange("p k b -> p b k"), axis=AX.X, op=ALU.add)
        nc.vector.tensor_scalar_mul(ex2[:], ex2[:], 1.0 / E)
        msq = ab.tile([1, B], F32, tag="msq")
        nc.vector.tensor_tensor(msq[:], mean[:], mean[:], op=ALU.mult)
        var = ab.tile([1, B], F32, tag="var")
        nc.vector.tensor_tensor(var[:], ex2[:], msq[:], op=ALU.subtract)
        sd = ab.tile([1, B], F32, tag="sd")
        nc.scalar.activation(sd[:], var[:], ACT_F.Sqrt, bias=eps_sb[:])
        rstd = ab.tile([1, B], F32, tag="rstd")
        nc.vector.reciprocal(rstd[:], sd[:])
        mu_b = ps.tile([128, B], F32, tag="ln")
        nc.tensor.matmul(mu_b[:], ones_row[:], mean[:], start=True, stop=True)
        rs_b = ps.tile([128, B], F32, tag="ln")
        nc.tensor.matmul(rs_b[:], ones_row[:], rstd[:], start=True, stop=True)
        xn = ab.tile([128, KC, B], F32, tag="xn")
        tmp = ab.tile([128, KC, B], F32, tag="lntmp")
        mu_bb = mu_b[:].rearrange("p (o b) -> p o b", o=1).broadcast_to([128, KC, B])
        rs_bb = rs_b[:].rearrange("p (o b) -> p o b", o=1).broadcast_to([128, KC, B])
        nc.vector.tensor_tensor(tmp[:], yT[:], mu_bb, op=ALU.subtract)
        nc.vector.tensor_tensor(xn[:], tmp[:], rs_bb, op=ALU.mult)
        if g is not None:
            for k in range(KC):
                if be is not None:
                    nc.vector.tensor_scalar(
                        xn[:, k, :], xn[:, k, :], g[:, k, :], be[:, k, :],
                        ALU.mult, ALU.add)
                else:
                    nc.vector.tensor_scalar_mul(xn[:, k, :], xn[:, k, :], g[:, k, :])
        elif be is not None:
            for k in range(KC):
                nc.vector.tensor_scalar_add(xn[:, k, :], xn[:, k, :], be[:, k, :])
        return xn

    def load_vec(name, n):
        # [n, 1] DRAM -> [128, n//128, 1] SBUF feature-major column
        t = wp.tile([128, n // 128, 1], F32, tag=name[:3])
        nc.sync.dma_start(t[:], h[name].ap().rearrange("(k p) o -> p k o", p=128))
        return t

    for l in range(L):
        wqkv_h = load_w(f"wqkv{l}", [E, 3 * ESH], F16)
        wo_h = load_w(f"wo{l}", [E, E], F16)
        w1_h = load_w(f"w1{l}", [E, FSH], F16)
        w2_h = load_w(f"w2{l}", [F, ESH], F16)
        if use_bias:
            bo = load_vec(f"bo{l}", E)
            bf2 = load_vec(f"bf2{l}", E)
            bf1 = load_vec(f"bf1{l}", FSH)
            bqkv_sb = wp.tile([B, 3 * ESH], F32, tag="bqkv")
            nc.sync.dma_start(bqkv_sb[:], h[f"bqkv{l}"].ap())
        g1 = load_vec(f"g1{l}", E) if use_affine else None
        be1 = load_vec(f"be1{l}", E) if use_affine else None
        g2 = load_vec(f"g2{l}", E) if use_affine else None
        be2 = load_vec(f"be2{l}", E) if use_affine else None

        # --- merged QKV: [32, 384] = x @ [Wq|Wk|Wv]_c, one fp16 stream
        xTh = ab.tile([128, KC, B], F16, tag="xth")
        nc.vector.tensor_copy(xTh[:], xT[:])
        qkv_ps = ps.tile([B, 3 * ESH], F32, tag="att")
        for k in range(KC):
            nc.tensor.matmul(qkv_ps[:], xTh[:, k, :], wqkv_h[:, k, :], start=(k == 0), stop=(k == KC - 1))
        qkv_sb = ab.tile([B, 3 * ESH], F16, tag="qkvs")
        if use_bias:
            nc.vector.tensor_tensor(qkv_sb[:], qkv_ps[:], bqkv_sb[:], op=ALU.add)
        else:
            nc.vector.tensor_copy(qkv_sb[:], qkv_ps[:])
        v_sb = qkv_sb[:, 2 * ESH:3 * ESH]
        qt_tp = ps.tile([128, B], F16, tag="qk")
        nc.tensor.transpose(qt_tp[:], qkv_sb[:, 0:ESH], ident16[:])
        kt_tp = ps.tile([128, B], F16, tag="qk")
        nc.tensor.transpose(kt_tp[:], qkv_sb[:, ESH:2 * ESH], ident16[:])
        qt_sb = ab.tile([128, B], F16, tag="qts")
        kt_sb = ab.tile([128, B], F16, tag="kts")
        nc.vector.tensor_copy(qt_sb[:], qt_tp[:])
        nc.vector.tensor_copy(kt_sb[:], kt_tp[:])

        # --- AllGather A: {QT_c, KT_c, V_c}
        agA_i = dr.tile([3, 128 * B], F16, tag="agAi")
        agA_o = dr.tile([NC, 3, 128 * B], F16, addr_space="Shared", tag="agAo")
        nc.sync.dma_start(agA_i[0, :].rearrange("(p b) -> p b", p=128), qt_sb[:])
        nc.sync.dma_start(agA_i[1, :].rearrange("(p b) -> p b", p=128), kt_sb[:])
        nc.sync.dma_start(agA_i[2, :].rearrange("(b f) -> b f", b=B), v_sb)
        nc.gpsimd.collective_compute(
            "AllGather", ALU.bypass, replica_groups=groups,
            ins=[agA_i.opt()], outs=[agA_o.opt()],
        )
        qtg = ab.tile([128, NSH, B], F16, tag="qtg")
        ktg = ab.tile([128, NSH, B], F16, tag="ktg")
        vg = ab.tile([B, NSH, 128], F16, tag="vg")
        nc.gpsimd.dma_start(qtg[:], agA_o[0:NSH, 0, :].rearrange("r (p b) -> p r b", p=128))
        nc.gpsimd.dma_start(ktg[:], agA_o[0:NSH, 1, :].rearrange("r (p b) -> p r b", p=128))
        nc.gpsimd.dma_start(vg[:], agA_o[0:NSH, 2, :].rearrange("r (b f) -> b r f", b=B))

        # --- scores + softmax (token-major [32, 32])
        sc_ps = ps.tile([B, B], F32, tag="att")
        for r in range(NSH):
            nc.tensor.matmul(sc_ps[:], qtg[:, r, :], ktg[:, r, :], start=(r == 0), stop=(r == NSH - 1))
        smax = ab.tile([B, 1], F32, tag="smax")
        nc.vector.reduce_max(smax[:], sc_ps[:], axis=AX.X)
        nmax = ab.tile([B, 1], F32, tag="nmax")
        nc.vector.tensor_scalar_mul(nmax[:], smax[:], -SCALE)
        attn = ab.tile([B, B], F32, tag="attn")
        rsum = ab.tile([B, 1], F32, tag="rsum")
        nc.scalar.activation(attn[:], sc_ps[:], ACT_F.Exp, bias=nmax[:], scale=SCALE,
                             accum_out=rsum[:])
        rinv = ab.tile([B, 1], F32, tag="rinv")
        nc.vector.reciprocal(rinv[:], rsum[:])
        attn_n = ab.tile([B, B], F16, tag="attn_n")
        nc.vector.tensor_scalar_mul(attn_n[:], attn[:], rinv[:])
        attnT = ab.tile([B, B], F16, tag="attnT")
        nc.vector.transpose(attnT[:], attn_n[:])

        # --- ao^T (feature-major, full E) then o = ao @ Wo (replicated Wo)
        aoT = ab.tile([128, KC, B], F16, tag="aoT")
        ao_ps = ps.tile([128, KC, B], F32, tag="qk")
        for m in range(KC):
            nc.tensor.matmul(ao_ps[:, m, :], vg[:, m, :], attnT[:], start=True, stop=True)
        nc.vector.tensor_copy(aoT[:], ao_ps[:])
        oT_ps = ps.tile([128, KC, B], F32, tag="qk")
        for m in range(KC):
            for k in range(KC):
                nc.tensor.matmul(oT_ps[:, m, :], wo_h[:, k, 128 * m:128 * (m + 1)],
                                 aoT[:, k, :], start=(k == 0), stop=(k == KC - 1))

        # --- residual + LN1
        y1 = ab.tile([128, KC, B], F32, tag="y1")
        nc.vector.tensor_tensor(y1[:], xT[:], oT_ps[:], op=ALU.add)
        if use_bias:
            for k in range(KC):
                nc.vector.tensor_scalar_add(y1[:, k, :], y1[:, k, :], bo[:, k, :])
        x1n = layernorm(y1, g1, be1)

        # --- FFN1: h_c = relu(x1n @ W1_c)  [32, 512]
        x1n_h = ab.tile([128, KC, B], F16, tag="x1nh")
        nc.vector.tensor_copy(x1n_h[:], x1n[:])
        hT_ps = ps.tile([128, FSH // 128, B], F32, tag="qk")
        for m in range(FSH // 128):
            for k in range(KC):
                nc.tensor.matmul(hT_ps[:, m, :], w1_h[:, k, 128 * m:128 * (m + 1)],
                                 x1n_h[:, k, :], start=(k == 0), stop=(k == KC - 1))
        hT = ab.tile([128, FSH // 128, B], F16, tag="hT")
        if use_bias:
            for m in range(FSH // 128):
                nc.vector.tensor_scalar(hT_ps[:, m, :], hT_ps[:, m, :],
                                        bf1[:, m, :], None, ALU.add)
        nc.vector.tensor_scalar_max(hT[:], hT_ps[:], 0.0)

        # --- AllGather B: hT_c
        agB_i = dr.tile([FSH * B], F16, tag="agBi")
        agB_o = dr.tile([NC, FSH * B], F16, addr_space="Shared", tag="agBo")
        nc.sync.dma_start(agB_i[:].rearrange("(c p b) -> p c b", c=4, p=128), hT[:])
        nc.gpsimd.collective_compute(
            "AllGather", ALU.bypass, replica_groups=groups,
            ins=[agB_i.opt()], outs=[agB_o.opt()],
        )
        hTg = ab.tile([128, F // 128, B], F16, tag="hTg")
        nc.gpsimd.dma_start(
            hTg[:], agB_o[0:NSH, :].rearrange("r (c p b) -> p (r c) b", c=4, p=128))

        # --- FFN2 shard (W-stationary): o2T_c = (h @ W2_c)^T  [128, 32]
        o2T_ps = ps.tile([128, B], F32, tag="qk")
        for t in range(F // 128):
            nc.tensor.matmul(o2T_ps[:], w2_h[:, t, :], hTg[:, t, :], start=(t == 0), stop=(t == F // 128 - 1))
        o2T = ab.tile([128, B], F16, tag="o2T")
        nc.vector.tensor_copy(o2T[:], o2T_ps[:])

        # --- AllGather C: o2T_c
        agC_i = dr.tile([128 * B], F16, tag="agCi")
        agC_o = dr.tile([NC, 128 * B], F16, addr_space="Shared", tag="agCo")
        nc.sync.dma_start(agC_i[:].rearrange("(p b) -> p b", p=128), o2T[:])
        nc.gpsimd.collective_compute(
            "AllGather", ALU.bypass, replica_groups=groups,
            ins=[agC_i.opt()], outs=[agC_o.opt()],
        )
        o2Tg = ab.tile([128, NSH, B], F16, tag="o2Tg")
        nc.gpsimd.dma_start(o2Tg[:], agC_o[0:NSH, :].rearrange("r (p b) -> p r b", p=128))

        # --- residual + LN2
        y2 = ab.tile([128, KC, B], F32, tag="y2")
        nc.vector.tensor_tensor(y2[:], x1n[:], o2Tg[:], op=ALU.add)
        if use_bias:
            for k in range(KC):
                nc.vector.tensor_scalar_add(y2[:, k, :], y2[:, k, :], bf2[:, k, :])
        xT = layernorm(y2, g2, be2)

    # --- classifier
    wc_sb = wp.tile([128, KC, C], F32, tag="wc")
    nc.sync.dma_start(wc_sb[:], h["wc"].ap().rearrange("(k p) n -> p k n", p=128))
    lg_ps = ps.tile([B, C], F32, tag="oo")
    for k in range(KC):
        nc.tensor.matmul(lg_ps[:], xT[:, k, :], wc_sb[:, k, :], start=(k == 0), stop=(k == KC - 1))
    lg_sb = ab.tile([B, C], F32, tag="lgs")
    if use_bias:
        bc_sb = wp.tile([B, C], F32, tag="bcs")
        nc.sync.dma_start(bc_sb[:], h["bc"].ap())
        nc.vector.tensor_tensor(lg_sb[:], lg_ps[:], bc_sb[:], op=ALU.add)
    else:
        nc.vector.tensor_copy(lg_sb[:], lg_ps[:])
    nc.sync.dma_start(h["out"].ap(), lg_sb[:])

    for p in reversed(ctxs):
        p.release()


def build(use_bias, use_affine):
    key = (use_bias, use_affine)
    if key in _CACHE:
        return _CACHE[key]
    nc = bacc.Bacc("TRN2", target_bir_lowering=False, debug=False, num_devices=NC)
    h = _declare(nc, use_bias, use_affine)
    with tile.TileContext(nc) as tc:
        _emit(tc, h, use_bias, use_affine)
    nc.compile()
    _CACHE[key] = (nc, h)
    return nc, h


def make_in_maps(inputs, use_bias, use_affine):
    inp = {k: np.ascontiguousarray(np.asarray(v, dtype=np.float32))
           if np.asarray(v).dtype != np.int32 and np.asarray(v).dtype != np.int64
           else np.asarray(v) for k, v in inputs.items()}
    ids = np.asarray(inputs["input_ids"])[0]
    embT = np.ascontiguousarray(inp["tok_emb"][ids].T)          # [768, 32]
    posT = np.ascontiguousarray(
        np.broadcast_to(inp["pos_emb"][0][:, None], (E, B)))
    in_maps = []
    for c in range(NC):
        sh = c % NSH
        m = {"embT": embT, "posT": posT, "wc": inp["Wc"]}
        for l in range(L):
            m[f"wqkv{l}"] = np.ascontiguousarray(np.concatenate([
                inp["Wq"][l][:, ESH * sh:ESH * (sh + 1)],
                inp["Wk"][l][:, ESH * sh:ESH * (sh + 1)],
                inp["Wv"][l][:, ESH * sh:ESH * (sh + 1)]], axis=1))
            m[f"wo{l}"] = np.ascontiguousarray(inp["Wo"][l])
            m[f"w1{l}"] = np.ascontiguousarray(inp["W1"][l][:, FSH * sh:FSH * (sh + 1)])
            m[f"w2{l}"] = np.ascontiguousarray(inp["W2"][l][:, ESH * sh:ESH * (sh + 1)])
            if use_bias:
                bqkv = np.concatenate([
                    inp["bq"][l][ESH * sh:ESH * (sh + 1)],
                    inp["bk"][l][ESH * sh:ESH * (sh + 1)],
                    inp["bv"][l][ESH * sh:ESH * (sh + 1)]])
                m[f"bqkv{l}"] = np.ascontiguousarray(
                    np.broadcast_to(bqkv[None, :], (B, 3 * ESH)))
                m[f"bo{l}"] = np.ascontiguousarray(inp["bo"][l][:, None])
                m[f"bf1{l}"] = np.ascontiguousarray(
                    inp["bf1"][l][FSH * sh:FSH * (sh + 1), None])
                m[f"bf2{l}"] = np.ascontiguousarray(inp["bf2"][l][:, None])
            if use_affine:
                m[f"g1{l}"] = np.ascontiguousarray(inp["g1"][l][:, None])
                m[f"be1{l}"] = np.ascontiguousarray(inp["beta1"][l][:, None])
                m[f"g2{l}"] = np.ascontiguousarray(inp["g2"][l][:, None])
                m[f"be2{l}"] = np.ascontiguousarray(inp["beta2"][l][:, None])
        if use_bias:
            m["bc"] = np.ascontiguousarray(np.broadcast_to(inp["bc"][None, :], (B, C)))
        in_maps.append(m)
    return in_maps


def _flags(inputs):
    z = lambda *names: all(not np.any(np.asarray(inputs[n])) for n in names)
    use_bias = not z("bq", "bk", "bv", "bo", "bf1", "bf2", "bc")
    use_affine = not (
        z("beta1", "beta2")
        and np.all(np.asarray(inputs["g1"]) == 1.0)
        and np.all(np.asarray(inputs["g2"]) == 1.0)
    )
    return use_bias, use_affine


def kernel(**inputs) -> np.ndarray:
    global LAST_RESULT
    use_bias, use_affine = _flags(inputs)
    nc, h = build(use_bias, use_affine)
    in_maps = make_in_maps(inputs, use_bias, use_affine)
    res = run_bass_kernel_spmd(nc, in_maps, core_ids=list(range(NC)))
    LAST_RESULT = res
    return np.asarray(res.results[0]["out"])



# revision 37
# speedup vs baseline: 1.0970x; 1.0970x over previous
"""Trainium2 Bass kernel for nn_BERTClassifier (batch-mixing attention BERT).

Only position 0 of the sequence reaches the output (attention mixes the batch
within a position; every op is independent across positions), so the network
reduces to a [32, 768] activation through 4 layers. The work is dominated by
streaming 113MB of weights, so we run 8-way tensor parallel with 96-wide
feature shards (384-wide FFN shards) and fp16 weights cast on the host.

Cross-core exchange uses direct SBUF-to-SBUF remote DMA instead of ncfw
collectives: each core issues one all-real `remote_dma_broadcast` per phase,
writing its shard into slot <own logical id> of every peer's recv tile
(canonical slot order). All 16 phase descriptor frames are pre-generated into
the SWDGE ring at kernel start (descriptors capture addresses, not data), so
a phase is just trigger_dma + a remote-semaphore wait (~3-4us). A tiny ncfw
warmup AllGather at the very start both absorbs core-launch skew and
guarantees every peer is past its semaphore-reset point before any remote
write lands.

Layout: residual stream feature-major [96 partitions, 8 chunks, 32 tokens],
chunk t = canonical features 96t..96t+95. Weights host-packed per layer into
two DRAM blobs ([96, 6144] for Wqkv|Wo|W1 and [128, 2304] for W2), one HWDGE
DMA each. fp32 residual/softmax/LayerNorm, fp16 matmuls with fp32 PSUM.

Self-contained: shapes hardcoded, no sibling imports.
"""
import sys
import types

import numpy as np

# If BASS_TRACE is set but the axon NTFF hook module is absent, bass_utils
# would crash importing antenv.axon_hooks. Provide a null hook so tracing
# degrades to a warning instead. (test.py installs the real hook first.)
try:
    from antenv import axon_hooks as _ah  # noqa: F401
except ImportError:
    try:
        import antenv as _antenv
        _mod = types.ModuleType("antenv.axon_hooks")
        _mod.get_axon_ntff_profile_hook = lambda: None
        _mod.set_axon_ntff_profile_hook = lambda h: None
        _antenv.axon_hooks = _mod
        sys.modules["antenv.axon_hooks"] = _mod
    except Exception:
        pass

import concourse.bass as bass
import concourse.bacc as bacc
import concourse.mybir as mybir
import concourse.tile as tile
from concourse.bass_utils import run_bass_kernel_spmd

F32 = mybir.dt.float32
F16 = mybir.dt.float16
I32 = mybir.dt.int32
AX = mybir.AxisListType
ALU = mybir.AluOpType
ACT_F = mybir.ActivationFunctionType

V, E, F, L, S, B, C = 30522, 768, 3072, 4, 512, 32, 2
import os as _os
L_RUN = int(_os.environ.get("KDBG_L", str(L)))  # debug: layers actually emitted
NC = 8
ESH = E // NC          # 96 feature shard
FSH = F // NC          # 384 ffn shard
KC = NC                # contraction chunks of 96 (canonical slot order)
SCALE = 1.0 / float(np.sqrt(E))
EPS = 1e-5

# payload free-elems (fp16) per phase
# All payloads uniform [128, 160] fp16 — the exact pregenerated-frame shape
# validated on HW (mixed sizes crash the SWDGE broadcast ring).
NA = 160   # sendA: QT|KT at [0:96, 0:64], V at [0:32, 64:160]
NB = 160   # sendB: oT_c at [0:96, 0:32]
NCx = 160  # sendC: hT [128, 3, 32] in cols 0:96
ND = 160   # sendD: o2T_c at [0:96, 0:32]

RDESTS = [(0, k) for k in range(NC)]

_CACHE = {}
LAST_RESULT = None  # BassKernelResults of the most recent run (for test.py)


def _declare(nc, use_bias, use_affine):
    h = {}
    h["x0"] = nc.dram_tensor("x0", [ESH, KC, B], F32, kind="ExternalInput")
    h["ident"] = nc.dram_tensor("ident", [B, B], F16, kind="ExternalInput")
    h["magic"] = nc.dram_tensor("magic", [1, B], I32, kind="ExternalInput")
    for l in range(L):
        h[f"p96_{l}"] = nc.dram_tensor(f"p96_{l}", [ESH, 8 * (288 + 96 + 384)],
                                       F16, kind="ExternalInput")
        h[f"p128_{l}"] = nc.dram_tensor(f"p128_{l}", [128, 24 * 96], F16,
                                        kind="ExternalInput")
        if use_bias:
            h[f"bqkv{l}"] = nc.dram_tensor(f"bqkv{l}", [B, 288], F32, kind="ExternalInput")
            h[f"bo{l}"] = nc.dram_tensor(f"bo{l}", [ESH, 1], F32, kind="ExternalInput")
            h[f"bf1{l}"] = nc.dram_tensor(f"bf1{l}", [128, 3], F32, kind="ExternalInput")
            h[f"bf2{l}"] = nc.dram_tensor(f"bf2{l}", [ESH, 1], F32, kind="ExternalInput")
        if use_affine:
            h[f"g1{l}"] = nc.dram_tensor(f"g1{l}", [ESH, KC], F32, kind="ExternalInput")
            h[f"be1{l}"] = nc.dram_tensor(f"be1{l}", [ESH, KC], F32, kind="ExternalInput")
            h[f"g2{l}"] = nc.dram_tensor(f"g2{l}", [ESH, KC], F32, kind="ExternalInput")
            h[f"be2{l}"] = nc.dram_tensor(f"be2{l}", [ESH, KC], F32, kind="ExternalInput")
    h["wc"] = nc.dram_tensor("wc", [ESH, KC * C], F16, kind="ExternalInput")
    if use_bias:
        h["bc"] = nc.dram_tensor("bc", [B, C], F32, kind="ExternalInput")
    h["out"] = nc.dram_tensor("out", [B, C], F32, kind="ExternalOutput")
    return h


def _emit(tc, h, use_bias, use_affine):
    nc = tc.nc
    ctxs = []

    def pool(*a, **k):
        p = tc.alloc_tile_pool(*a, **k)
        ctxs.append(p)
        return p

    const = pool(name="const", bufs=1)
    wp = pool(name="wts", bufs=2)
    ab = pool(name="act", bufs=1)
    cm = pool(name="comm", bufs=1)
    ps = pool(name="ps", bufs=1, space="PSUM")
    dr = pool(name="dram", bufs=1, space="DRAM")

    # ---- ncfw warmup AllGather: launch-skew barrier + collective init.
    wu_i = dr.tile([64], F32, tag="wui")
    wu_o = dr.tile([NC, 64], F32, addr_space="Shared", tag="wuo")
    wu_sb = const.tile([1, 64], F32, tag="wus")
    nc.vector.memset(wu_sb[:], 0.0)
    nc.sync.dma_start(wu_i[:].rearrange("(p n) -> p n", p=1), wu_sb[:])
    nc.gpsimd.collective_compute(
        "AllGather", ALU.bypass, replica_groups=[list(range(NC))],
        ins=[wu_i.opt()], outs=[wu_o.opt()],
    )
    wu_chk = const.tile([1, 64], F32, tag="wuchk")
    nc.sync.dma_start(wu_chk[:], wu_o[0:1, :])

    # ---- constants
    ones_col = const.tile([ESH, 1], F32)
    nc.vector.memset(ones_col[:], 1.0)
    ones_row = const.tile([1, ESH], F32)
    nc.vector.memset(ones_row[:], 1.0)
    eps_sb = const.tile([1, 1], F32)
    nc.vector.memset(eps_sb[:], EPS)
    ident16 = const.tile([B, B], F16)
    nc.sync.dma_start(ident16[:], h["ident"].ap())
    magic_sb = const.tile([1, B], I32, tag="magic")
    nc.sync.dma_start(magic_sb[:], h["magic"].ap())

    # ---- weight packs (HWDGE, double-buffered per layer)
    wc_sb = wp.tile([ESH, KC * C], F16, tag="wc")
    nc.sync.dma_start(wc_sb[:], h["wc"].ap())
    x0_sb = ab.tile([ESH, KC, B], F32, tag="x0")
    nc.sync.dma_start(x0_sb[:], h["x0"].ap())

    def load_packs(l):
        t96 = wp.tile([ESH, 8 * (288 + 96 + 384)], F16, tag="p96",
                      name=f"p96t_{l}")
        nc.sync.dma_start(t96[:], h[f"p96_{l}"].ap())
        t128 = wp.tile([128, 24 * 96], F16, tag="p128", name=f"p128t_{l}")
        nc.sync.dma_start(t128[:], h[f"p128_{l}"].ap())
        wqkv = t96[:, 0:2304].rearrange("p (k n) -> p k n", n=288)
        wo = t96[:, 2304:3072].rearrange("p (k n) -> p k n", n=96)
        w1 = t96[:, 3072:6144].rearrange("p (k n) -> p k n", n=384)
        w2 = t128[:, :].rearrange("p (k n) -> p k n", n=96)
        return wqkv, wo, w1, w2

    # ---- per-phase comm buffers (all distinct; canonical slot order)
    sends, recvs, recv_sems = [], [], []
    for l in range(L_RUN):
        for phn, n in (("A", NA), ("B", NB), ("C", NCx), ("D", ND)):
            sends.append(cm.tile([128, n], F16, tag=f"s{phn}{l}",
                                 name=f"send_{phn}{l}"))
            # one-time zero of the garbage regions the payload writers skip
            nc.vector.memset(sends[-1][:], 0.0)
            recvs.append(cm.tile([128, NC, n], F16, tag=f"r{phn}{l}",
                                 name=f"recv_{phn}{l}"))
            recv_sems.append(nc.alloc_semaphore(f"rsem_{phn}{l}"))
    prep_sem = nc.alloc_semaphore("prep_sem")
    send_sem = nc.alloc_semaphore("send_sem")
    anchor = cm.tile([1, 16], F16, tag="anchor")

    # ---- pre-generate broadcast frames in batches (addresses only; data is
    # read at trigger time). The Pool drain at critical exit would flush the
    # untriggered ring, so skip it. After each batch, drop bass's
    # prep->first-trigger dep inheritance (it would make the next trigger
    # wait on every batched phase's send producer: a cycle). Ordering is
    # explicit: prep_sem, per-phase send anchors, engine-FIFO waits.
    prep_done = [0]

    def pregen_batch(lo, hi):
        with tc.tile_critical(no_gpsimd_drain=True):
            rv = nc.gpsimd.partition_id()
            for c in nc.gpsimd.Switch(rv, NC):
                for i in range(lo, hi):
                    nc.gpsimd.remote_dma_broadcast(
                        recvs[i][:, c, :], sends[i][:],
                        remote_sem=recv_sems[i], local_sem=send_sem,
                        rdests=RDESTS,
                    ).then_inc(prep_sem, 1)
            prep_done[0] += hi - lo
            nc.gpsimd.wait_ge(prep_sem, prep_done[0])
        nc.gpsimd._pending_untriggered_insts[0].clear()

    pregen_batch(0, min(4, len(sends)))

    phase_idx = [0]

    def gather_phase_pe(i, n):
        """Fire frame i; vector-side wait, then a 1-elem pad write into the
        recv tile so it becomes a critical OUTPUT: downstream readers (on any
        engine) are ordered after the wait via post_crit."""
        with tc.tile_critical(no_gpsimd_drain=True):
            nc.vector.tensor_copy(anchor[0:1, i:i + 1], sends[i][0:1, 0:1])
            nc.gpsimd.trigger_dma(count=1)
            nc.vector.wait_ge(recv_sems[i], 2 * NC)
            nc.vector.memset(recvs[i][96:97, 0, n - 1:n], 0.0)
        return recvs[i]

    def gather_phase_add(i, out_f32, a_f32):
        """Fire frame i; vector-side wait, then out = a + recv chunks
        inside the critical (FIFO-ordered after the wait)."""
        with tc.tile_critical(no_gpsimd_drain=True):
            nc.vector.tensor_copy(anchor[0:1, i:i + 1], sends[i][0:1, 0:1])
            nc.gpsimd.trigger_dma(count=1)
            nc.vector.wait_ge(recv_sems[i], 2 * NC)
            nc.vector.tensor_tensor(out_f32[:], a_f32[:],
                                    recvs[i][0:ESH, :, 0:B], op=ALU.add)

    def layernorm(ysq, g=None, be=None):
        # caller wrote y into ysq[:, 0:KC, :]; square into the second half so
        # ONE column-sum matmul covers both sums (N=512 = one PSUM bank)
        yT = ysq[:, 0:KC, :]
        nc.vector.tensor_tensor(ysq[:, KC:2 * KC, :], yT, yT, op=ALU.mult)
        s12_ps = ps.tile([1, 2 * KC, B], F32, tag="ln")
        nc.tensor.matmul(s12_ps[:], ones_col[:], ysq[:], start=True, stop=True)
        mean = ab.tile([1, B], F32, tag="mean")
        nc.vector.tensor_reduce(
            mean[:], s12_ps[:, 0:KC, :].rearrange("p k b -> p b k"),
            axis=AX.X, op=ALU.add)
        nc.vector.tensor_scalar_mul(mean[:], mean[:], 1.0 / E)
        ex2 = ab.tile([1, B], F32, tag="ex2")
        nc.vector.tensor_reduce(
            ex2[:], s12_ps[:, KC:2 * KC, :].rearrange("p k b -> p b k"),
            axis=AX.X, op=ALU.add)
        nc.vector.tensor_scalar_mul(ex2[:], ex2[:], 1.0 / E)
        msq = ab.tile([1, B], F32, tag="msq")
        nc.vector.tensor_tensor(msq[:], mean[:], mean[:], op=ALU.mult)
        var = ab.tile([1, B], F32, tag="var")
        nc.vector.tensor_tensor(var[:], ex2[:], msq[:], op=ALU.subtract)
        # rstd = 1/sqrt(var+eps) on DVE (bit-trick + 2 Newton steps) so the
        # ACT engine keeps its Exp table resident (no Sqrt table thrash)
        veps = ab.tile([1, B], F32, tag="veps")
        nc.vector.tensor_scalar_add(veps[:], var[:], EPS)
        r0i = ab.tile([1, B], I32, tag="r0i")
        nc.vector.tensor_scalar(r0i[:], veps[:].bitcast(I32), 1, None,
                                ALU.logical_shift_right)
        nc.vector.tensor_tensor(r0i[:], magic_sb[:], r0i[:], op=ALU.subtract)
        r = r0i[:].bitcast(F32)
        rstd = ab.tile([1, B], F32, tag="rstd")
        t1 = ab.tile([1, B], F32, tag="lnt1")
        for _ in range(2):
            nc.vector.tensor_tensor(t1[:], r, r, op=ALU.mult)
            nc.vector.tensor_tensor(t1[:], t1[:], veps[:], op=ALU.mult)
            nc.vector.tensor_scalar(t1[:], t1[:], -0.5, 1.5, ALU.mult, ALU.add)
            nc.vector.tensor_tensor(rstd[:], t1[:], r, op=ALU.mult)
            r = rstd[:]
        mu_b = ps.tile([ESH, B], F32, tag="tp")
        nc.tensor.matmul(mu_b[:], ones_row[:], mean[:], start=True, stop=True)
        rs_b = ps.tile([ESH, B], F32, tag="tp")
        nc.tensor.matmul(rs_b[:], ones_row[:], rstd[:], start=True, stop=True)
        xn = ab.tile([ESH, KC, B], F32, tag="xn")
        tmp = ab.tile([ESH, KC, B], F32, tag="lntmp")
        mu_bb = mu_b[:].rearrange("p (o b) -> p o b", o=1).broadcast_to([ESH, KC, B])
        rs_bb = rs_b[:].rearrange("p (o b) -> p o b", o=1).broadcast_to([ESH, KC, B])
        nc.vector.tensor_tensor(tmp[:], yT, mu_bb, op=ALU.subtract)
        nc.vector.tensor_tensor(xn[:], tmp[:], rs_bb, op=ALU.mult)
        if g is not None:
            for k in range(KC):
                if be is not None:
                    nc.vector.tensor_scalar(
                        xn[:, k, :], xn[:, k, :], g[:, k:k + 1], be[:, k:k + 1],
                        ALU.mult, ALU.add)
                else:
                    nc.vector.tensor_scalar_mul(xn[:, k, :], xn[:, k, :], g[:, k:k + 1])
        elif be is not None:
            for k in range(KC):
                nc.vector.tensor_scalar_add(xn[:, k, :], xn[:, k, :], be[:, k:k + 1])
        return xn

    xT = x0_sb
    for l in range(L_RUN):
        wqkv_l, wo_l, w1_l, w2_l = load_packs(l)
        if use_bias:
            bqkv_sb = wp.tile([B, 288], F32, tag=f"bqkv{l}")
            nc.sync.dma_start(bqkv_sb[:], h[f"bqkv{l}"].ap())
            bo_sb = wp.tile([ESH, 1], F32, tag=f"bo{l}")
            nc.sync.dma_start(bo_sb[:], h[f"bo{l}"].ap())
            bf1_sb = wp.tile([128, 3], F32, tag=f"bf1{l}")
            nc.sync.dma_start(bf1_sb[:], h[f"bf1{l}"].ap())
            bf2_sb = wp.tile([ESH, 1], F32, tag=f"bf2{l}")
            nc.sync.dma_start(bf2_sb[:], h[f"bf2{l}"].ap())
        if use_affine:
            g1_sb = wp.tile([ESH, KC], F32, tag=f"g1{l}")
            nc.sync.dma_start(g1_sb[:], h[f"g1{l}"].ap())
            be1_sb = wp.tile([ESH, KC], F32, tag=f"be1{l}")
            nc.sync.dma_start(be1_sb[:], h[f"be1{l}"].ap())
            g2_sb = wp.tile([ESH, KC], F32, tag=f"g2{l}")
            nc.sync.dma_start(g2_sb[:], h[f"g2{l}"].ap())
            be2_sb = wp.tile([ESH, KC], F32, tag=f"be2{l}")
            nc.sync.dma_start(be2_sb[:], h[f"be2{l}"].ap())

        xTh = ab.tile([ESH, KC, B], F16, tag=f"xth{l}")
        nc.vector.tensor_copy(xTh[:], xT[:])

        # --- QKV shard: [32, 288] = x @ [Wq|Wk|Wv]_c
        qkv_ps = ps.tile([B, 288], F32, tag="att")
        for t in range(KC):
            nc.tensor.matmul(qkv_ps[:], xTh[:, t, :], wqkv_l[:, t, :],
                             start=(t == 0), stop=(t == KC - 1))
        qkv_sb = ab.tile([B, 288], F16, tag="qkvs")
        if use_bias:
            nc.vector.tensor_tensor(qkv_sb[:], qkv_ps[:], bqkv_sb[:], op=ALU.add)
        else:
            nc.vector.tensor_copy(qkv_sb[:], qkv_ps[:])

        # --- sendA: QT, KT (PE transpose), V (direct copy, token-major)
        sA = sends[4 * l + 0]
        qt_tp = ps.tile([ESH, B], F16, tag="tp")
        nc.tensor.transpose(qt_tp[:], qkv_sb[:, 0:ESH], ident16[:])
        nc.vector.tensor_copy(sA[0:ESH, 0:B], qt_tp[:])
        kt_tp = ps.tile([ESH, B], F16, tag="tp")
        nc.tensor.transpose(kt_tp[:], qkv_sb[:, ESH:2 * ESH], ident16[:])
        nc.vector.tensor_copy(sA[0:ESH, B:2 * B], kt_tp[:])
        nc.vector.tensor_copy(sA[0:B, 64:160], qkv_sb[:, 2 * ESH:288])

        rA = gather_phase_pe(4 * l + 0, NA)  # [128, 8, 160]

        # --- scores + softmax (token-major [32, 32])
        sc_ps = ps.tile([B, B], F32, tag="att")
        for s in range(NC):
            nc.tensor.matmul(sc_ps[:], rA[0:ESH, s, 0:B], rA[0:ESH, s, B:2 * B],
                             start=(s == 0), stop=(s == NC - 1))
        # scores*SCALE is bounded ~|1.2| for LN'd activations: exp() needs
        # no max-subtraction (fp32 overflows only past 88)
        attn = ab.tile([B, B], F32, tag="attn")
        rsum = ab.tile([B, 1], F32, tag="rsum")
        nc.scalar.activation(attn[:], sc_ps[:], ACT_F.Exp,
                             scale=SCALE, accum_out=rsum[:])
        rinv = ab.tile([B, 1], F32, tag="rinv")
        nc.vector.reciprocal(rinv[:], rsum[:])
        attn_n = ab.tile([B, B], F16, tag="attn_n")
        nc.vector.tensor_scalar_mul(attn_n[:], attn[:], rinv[:])
        attnT = ab.tile([B, B], F16, tag="attnT")
        nc.vector.transpose(attnT[:], attn_n[:])

        # --- aoT feature-major [96, 8, 32]: aoT_s = V_s^T-contraction
        ao_ps = ps.tile([ESH, NC, B], F32, tag="ao")
        for s in range(NC):
            nc.tensor.matmul(ao_ps[:, s, :], rA[0:B, s, 64:160], attnT[:],
                             start=True, stop=True)
        aoT = ab.tile([ESH, NC, B], F16, tag="aoT")
        nc.vector.tensor_copy(aoT[:], ao_ps[:])

        # --- oT_c = (ao @ Wo_c)^T  [96, 32]
        oT_ps = ps.tile([ESH, B], F32, tag="oT")
        for s in range(NC):
            nc.tensor.matmul(oT_ps[:], wo_l[:, s, :], aoT[:, s, :],
                             start=(s == 0), stop=(s == NC - 1))
        sB = sends[4 * l + 1]
        if use_bias:
            nc.vector.tensor_scalar(sB[0:ESH, 0:B], oT_ps[:], bo_sb[:, 0:1], None, ALU.add)
        else:
            nc.vector.tensor_copy(sB[0:ESH, 0:B], oT_ps[:])

        # --- residual + LN1 (o chunks arrive canonically ordered)
        y1 = ab.tile([ESH, 2 * KC, B], F32, tag="y1")
        gather_phase_add(4 * l + 1, y1[:, 0:KC, :], xT)
        if 4 * (l + 1) < len(sends):
            # prep next layer's 4 frames here: the gpsimd descgen overlaps
            # LN1+FFN1 on vector/PE; pending frames stay <= 6
            pregen_batch(4 * (l + 1), 4 * (l + 1) + 4)
        x1n = layernorm(y1, g1_sb if use_affine else None,
                        be1_sb if use_affine else None)
        x1nh = ab.tile([ESH, KC, B], F16, tag="x1nh")
        nc.vector.tensor_copy(x1nh[:], x1n[:])

        # --- FFN1 shard: hT = relu(x1n @ W1_c)^T  [128, 3, 32]
        hT_ps = ps.tile([128, 3, B], F32, tag="hT")
        for m in range(3):
            for t in range(KC):
                nc.tensor.matmul(hT_ps[:, m, :], w1_l[:, t, 128 * m:128 * (m + 1)],
                                 x1nh[:, t, :], start=(t == 0), stop=(t == KC - 1))
        sC = sends[4 * l + 2]
        sC_v = sC[:, 0:96].rearrange("p (k b) -> p k b", k=3)
        for m in range(3):
            if use_bias:
                nc.vector.tensor_scalar(hT_ps[:, m, :], hT_ps[:, m, :],
                                        bf1_sb[:, m:m + 1], None, ALU.add)
            nc.vector.tensor_scalar_max(sC_v[:, m, :], hT_ps[:, m, :], 0.0)

        rC = gather_phase_pe(4 * l + 2, NCx)  # [128, 8, 97]
        rC_v = rC[:, :, 0:96].rearrange("p s (k b) -> p s k b", k=3)

        # --- FFN2 shard: o2T_c = (h @ W2_c)^T  [96, 32]
        o2_ps = ps.tile([ESH, B], F32, tag="o2")
        for s in range(NC):
            for i in range(3):
                nc.tensor.matmul(o2_ps[:], w2_l[:, 3 * s + i, :], rC_v[:, s, i, :],
                                 start=(s == 0 and i == 0),
                                 stop=(s == NC - 1 and i == 2))
        sD = sends[4 * l + 3]
        if use_bias:
            nc.vector.tensor_scalar(sD[0:ESH, 0:B], o2_ps[:], bf2_sb[:, 0:1], None, ALU.add)
        else:
            nc.vector.tensor_copy(sD[0:ESH, 0:B], o2_ps[:])

        # --- residual + LN2
        y2 = ab.tile([ESH, 2 * KC, B], F32, tag="y2")
        gather_phase_add(4 * l + 3, y2[:, 0:KC, :], x1n)
        xT = layernorm(y2, g2_sb if use_affine else None,
                       be2_sb if use_affine else None)

    # --- classifier (replicated; harness reads core 0)
    xfh = ab.tile([ESH, KC, B], F16, tag="xfh")
    nc.vector.tensor_copy(xfh[:], xT[:])
    wc_v = wc_sb[:].rearrange("p (k n) -> p k n", n=C)
    lg_ps = ps.tile([B, C], F32, tag="oo")
    for t in range(KC):
        nc.tensor.matmul(lg_ps[:], xfh[:, t, :], wc_v[:, t, :],
                         start=(t == 0), stop=(t == KC - 1))
    lg_sb = ab.tile([B, C], F32, tag="lgs")
    if use_bias:
        bc_sb = wp.tile([B, C], F32, tag="bcs")
        nc.sync.dma_start(bc_sb[:], h["bc"].ap())
        nc.vector.tensor_tensor(lg_sb[:], lg_ps[:], bc_sb[:], op=ALU.add)
    else:
        nc.vector.tensor_copy(lg_sb[:], lg_ps[:])
    nc.sync.dma_start(h["out"].ap(), lg_sb[:])

    for p in reversed(ctxs):
        p.release()


def build(use_bias, use_affine):
    key = (use_bias, use_affine)
    if key in _CACHE:
        return _CACHE[key]
    nc = bacc.Bacc("TRN2", target_bir_lowering=False, debug=False,
                   num_devices=NC, dynamic_dma_scratch_size=49152)
    h = _declare(nc, use_bias, use_affine)
    with tile.TileContext(nc) as tc:
        _emit(tc, h, use_bias, use_affine)
    nc.compile()
    _CACHE[key] = (nc, h)
    return nc, h


def make_in_maps(inputs, use_bias, use_affine):
    f32 = lambda x: np.asarray(x, dtype=np.float32)
    ids = np.asarray(inputs["input_ids"])[0]
    x0 = f32(inputs["tok_emb"])[ids] + f32(inputs["pos_emb"])[0][None, :]  # [32, 768]
    x0T = np.ascontiguousarray(
        x0.T.reshape(KC, ESH, B).transpose(1, 0, 2))  # [96, 8, 32]

    def fm96(w):  # [768, n] -> [96, 8, n] feature-major chunks of 96
        n = w.shape[1]
        return w.reshape(KC, ESH, n).transpose(1, 0, 2)

    ident = np.eye(B, dtype=np.float16)
    in_maps = []
    for c in range(NC):
        es = slice(ESH * c, ESH * (c + 1))
        fs = slice(FSH * c, FSH * (c + 1))
        m = {"x0": x0T, "ident": ident,
             "magic": np.full((1, B), 0x5f3759df, dtype=np.int32)}
        for l in range(L):
            wqkv = np.concatenate([
                f32(inputs["Wq"][l])[:, es],
                f32(inputs["Wk"][l])[:, es],
                f32(inputs["Wv"][l])[:, es]], axis=1)          # [768, 288]
            wo = f32(inputs["Wo"][l])[:, es]                   # [768, 96]
            w1 = f32(inputs["W1"][l])[:, fs]                   # [768, 384]
            p96 = np.concatenate([
                fm96(wqkv).reshape(ESH, -1),
                fm96(wo).reshape(ESH, -1),
                fm96(w1).reshape(ESH, -1)], axis=1)            # [96, 6144]
            m[f"p96_{l}"] = np.ascontiguousarray(p96.astype(np.float16))
            w2 = f32(inputs["W2"][l])[:, es]                   # [3072, 96]
            p128 = w2.reshape(24, 128, 96).transpose(1, 0, 2).reshape(128, -1)
            m[f"p128_{l}"] = np.ascontiguousarray(p128.astype(np.float16))
            if use_bias:
                bqkv = np.concatenate([
                    f32(inputs["bq"][l])[es], f32(inputs["bk"][l])[es],
                    f32(inputs["bv"][l])[es]])
                m[f"bqkv{l}"] = np.ascontiguousarray(
                    np.broadcast_to(bqkv[None, :], (B, 288)).astype(np.float32))
                m[f"bo{l}"] = np.ascontiguousarray(f32(inputs["bo"][l])[es][:, None])
                m[f"bf1{l}"] = np.ascontiguousarray(
                    f32(inputs["bf1"][l])[fs].reshape(3, 128).T)
                m[f"bf2{l}"] = np.ascontiguousarray(f32(inputs["bf2"][l])[es][:, None])
            if use_affine:
                m[f"g1{l}"] = np.ascontiguousarray(fm96(f32(inputs["g1"][l])[:, None])[:, :, 0])
                m[f"be1{l}"] = np.ascontiguousarray(fm96(f32(inputs["beta1"][l])[:, None])[:, :, 0])
                m[f"g2{l}"] = np.ascontiguousarray(fm96(f32(inputs["g2"][l])[:, None])[:, :, 0])
                m[f"be2{l}"] = np.ascontiguousarray(fm96(f32(inputs["beta2"][l])[:, None])[:, :, 0])
        m["wc"] = np.ascontiguousarray(
            fm96(f32(inputs["Wc"])).reshape(ESH, -1).astype(np.float16))
        if use_bias:
            m["bc"] = np.ascontiguousarray(
                np.broadcast_to(f32(inputs["bc"])[None, :], (B, C)).astype(np.float32))
        in_maps.append(m)
    return in_maps


def _flags(inputs):
    z = lambda *names: all(not np.any(np.asarray(inputs[n])) for n in names)
    use_bias = not z("bq", "bk", "bv", "bo", "bf1", "bf2", "bc")
    use_affine = not (
        z("beta1", "beta2")
        and np.all(np.asarray(inputs["g1"]) == 1.0)
        and np.all(np.asarray(inputs["g2"]) == 1.0)
    )
    return use_bias, use_affine


def kernel(**inputs) -> np.ndarray:
    global LAST_RESULT
    use_bias, use_affine = _flags(inputs)
    nc, h = build(use_bias, use_affine)
    in_maps = make_in_maps(inputs, use_bias, use_affine)
    res = run_bass_kernel_spmd(nc, in_maps, core_ids=list(range(NC)))
    LAST_RESULT = res
    return np.asarray(res.results[0]["out"])
